# revision 50
# baseline (speedup 1.0000x reference)
"""Trainium2 Bass kernel for nn_MemStream (scatter_memory).

Per NeuronCore (batch sharded 32 rows/core, memory replicated; 8 cores):
  1. Host folds the input normalization into the encoder weights
     (W1p = inv_std[:,None]*W1, b1p = b1 - (mean*inv_std)@W1), so the device
     computes encT = tanh(W1p^T @ xT_shard + b1p) with one PE matmul + ACT.
  2. `memory` arrives pre-cast to bf16; PE transposes it into memT[h] =
     [128 (d-half), 4096 (m)].
  3. L1 distance via the identity |D| = 2*relu(D) - D:
       dist[b,m] = 2*sum_d relu(memT - encT[:,b]) - Sm[m] + Se[b]
     with Sm (memory row sums) from host and Se from a tiny PE matmul.
     relu(memT - encT[:,b]) units are produced by BOTH the vector engine
     (fused tensor_scalar subtract+max, bf16 4x / fp8 2x) and the scalar
     engine (activation Relu with bias=-encT), greedily load-balanced.
     PE reduces over d with a sliding one-hot stationary (value 2.0)
     accumulating into PSUM; 12 of 32 rows use fp8e4 units with DoubleRow
     matmuls that contract both d-halves in one pass (half the PE cost).
     The "-Sm" term is folded into PSUM by a K=1 matmul with a -1 row.
  4. DVE min-reduces PSUM per bank; loss[b] = min + Se[b].
  5. Host performs the sequential circular-buffer scatter (<=256 rows,
     trivially serial) on the full-precision enc rows.

Engine balance (cost model, per core): PE ~86us, DVE ~79us, ACT ~76us,
total ~110us. Self-contained: all shapes hardcoded; no sibling imports.
"""

from contextlib import ExitStack

import numpy as np

import concourse.bass as bass
import concourse.tile as tile
from concourse import bacc, mybir
from concourse.bass_utils import run_bass_kernel_spmd

B, D_IN, D_OUT, M = 256, 128, 256, 4096
NCORES = 8
BC = B // NCORES          # 32 batch rows per core
NCHUNK = M // 128         # 32 memory chunks of 128 rows
NPASS = 2                 # m-halves processed per PSUM residency
HALF = M // NPASS         # 2048
BETA = 50.0

f32 = mybir.dt.float32
bf16 = mybir.dt.bfloat16
fp8e4 = mybir.dt.float8e4
NB8 = 3                   # of every 8 batch rows, this many take the fp8 DoubleRow path
Alu = mybir.AluOpType
Act = mybir.ActivationFunctionType

_CACHE = {}


def _build_bass():
    # NOTE: every tile consumed by the PE (tensor engine) must be produced by
    # the vector engine (DVE) so PE instructions carry at most ONE sync wait:
    # walrus codegen allows a single wait on the implicit LDWEIGHTS of
    # matmul/transpose (S3_LW_STRUCT "Too many sync wait commands").
    nc = bacc.Bacc("TRN2", target_bir_lowering=False, debug=False)
    xt = nc.dram_tensor("xt", [D_IN, BC], f32, kind="ExternalInput")
    w1p = nc.dram_tensor("w1p", [D_IN, D_OUT], f32, kind="ExternalInput")
    b1p2 = nc.dram_tensor("b1p2", [128, 2], f32, kind="ExternalInput")
    identin = nc.dram_tensor("identin", [128, 128], bf16, kind="ExternalInput")
    mem = nc.dram_tensor("mem", [M, D_OUT], bf16, kind="ExternalInput")
    smin = nc.dram_tensor("sm", [1, M], bf16, kind="ExternalInput")
    loss_out = nc.dram_tensor("loss", [BC, 1], f32, kind="ExternalOutput")
    enc_out = nc.dram_tensor("enc_t", [2, 128, BC], f32, kind="ExternalOutput")

    with tile.TileContext(nc) as tc, ExitStack() as ctx:
        singles = ctx.enter_context(tc.tile_pool(name="singles", bufs=1))
        memsb = ctx.enter_context(tc.tile_pool(name="memsb", bufs=8))
        absp = ctx.enter_context(tc.tile_pool(name="absdiff", bufs=6))
        outsp = ctx.enter_context(tc.tile_pool(name="outs", bufs=1))
        setup_psum = ExitStack()
        psum_enc = setup_psum.enter_context(
            tc.tile_pool(name="psum_enc", bufs=2, space="PSUM")
        )
        psum_tp = setup_psum.enter_context(
            tc.tile_pool(name="psum_tp", bufs=2, space="PSUM")
        )

        # --- small inputs ---
        xt_sb = singles.tile([D_IN, BC], f32)
        nc.sync.dma_start(xt_sb[:], xt[:, :])
        w1p_sb = singles.tile([D_IN, D_OUT], f32)
        nc.sync.dma_start(w1p_sb[:], w1p[:, :])
        b1p2_sb = singles.tile([128, 2], f32)
        nc.sync.dma_start(b1p2_sb[:], b1p2[:, :])
        ident = singles.tile([128, 128], bf16)
        nc.sync.dma_start(ident[:], identin[:, :])

        # one-hot sliding strip: column 31 is 2.0 (folds the *2 of
        # |D| = 2*relu(D) - D into the PE reduction); strip[:, 31-b:63-b] is
        # the [128, 32] stationary whose only nonzero column is b.
        strip = singles.tile([128, 63], bf16)
        nc.vector.memset(strip[:], 0.0)
        nc.vector.memset(strip[:, 31:32], 2.0)
        # paired variant for fp8 DoubleRow matmuls (both d-halves per pass)
        strip2 = singles.tile([128, 2, 64], fp8e4, name="strip2")
        nc.vector.memset(strip2[:], 0.0)
        nc.vector.memset(strip2[:, :, 31:32], 2.0)
        ones_col = singles.tile([128, 1], f32)
        nc.vector.memset(ones_col[:], 1.0)
        # -1 row: stationary for the K=1 matmul folding "- Sm[m]" into PSUM
        negrow = singles.tile([1, BC], bf16)
        nc.vector.memset(negrow[:], -1.0)
        sm_sb = singles.tile([1, M], bf16)
        nc.sync.dma_start(sm_sb[:], smin[:, :])

        # --- encoder: encT[h] = tanh(W1p[:, h]^T @ xT + b1p[h]) ---
        encT_f32 = singles.tile([128, 2, BC], f32)
        for h in range(2):
            pe = psum_enc.tile([128, BC], f32, tag="pe", name="pe")
            nc.tensor.matmul(
                pe[:], w1p_sb[:, h * 128 : (h + 1) * 128], xt_sb[:],
                start=True, stop=True,
            )
            nc.scalar.activation(
                encT_f32[:, h, :], pe[:], Act.Tanh,
                bias=b1p2_sb[:, h : h + 1], scale=1.0,
            )
            nc.sync.dma_start(enc_out[h], encT_f32[:, h, :])
        # Se[b] = sum_d encT[d, b] (for the relu identity correction term)
        pse = psum_enc.tile([BC, 1], f32, tag="pe", name="pse")
        for h in range(2):
            nc.tensor.matmul(
                pse[:], encT_f32[:, h, :], ones_col[:],
                start=(h == 0), stop=(h == 1),
            )
        se_sb = outsp.tile([BC, 1], f32)
        nc.vector.tensor_copy(se_sb[:], pse[:])
        # negated encT: bias operand for ACT-produced relu-diff units
        neg_encT = singles.tile([128, 2, BC], f32, name="neg_encT")
        nc.vector.tensor_scalar(
            neg_encT[:], encT_f32[:], -1.0, None, op0=Alu.mult
        )

        # --- memory: DMA (bf16, pre-cast on host) -> PE transpose -> ACT copy
        # memT[h] = [128 (d-half), 4096 (m)] bf16
        mem_r = mem.rearrange("(c p) d -> p c d", p=128)
        memT = [
            singles.tile([128, M], bf16, tag=f"memT{h}", name=f"memT{h}")
            for h in range(2)
        ]
        for cg in range(8):  # chunk groups of 4 => m-cols of 512
            ms = memsb.tile([128, 4, D_OUT], bf16, tag="ms", name="ms")
            nc.sync.dma_start(ms[:], mem_r[:, cg * 4 : (cg + 1) * 4, :])
            for h in range(2):
                pt = psum_tp.tile([128, 512], bf16)
                for j in range(4):
                    c = cg * 4 + j
                    nc.tensor.transpose(
                        pt[:, j * 128 : (j + 1) * 128],
                        ms[:, j, h * 128 : (h + 1) * 128],
                        ident[:],
                    )
                # alternate PSUM->SBUF copies across ACT and DVE so neither
                # serializes the memT pipeline; cg 0 stays on DVE for both
                # halves (ACT is busy with its table load + tanh at startup)
                if h == 0 and cg > 0:
                    nc.scalar.copy(
                        memT[h][:, cg * 512 : (cg + 1) * 512], pt[:]
                    )
                else:
                    nc.vector.tensor_copy(
                        memT[h][:, cg * 512 : (cg + 1) * 512], pt[:]
                    )

        # release setup PSUM (enc + transpose pools) so the dist PSUM can be
        # double-buffered across the two m-half passes
        setup_psum.close()
        psum_dist = ctx.enter_context(
            tc.tile_pool(name="psum_dist", bufs=2, space="PSUM")
        )

        # --- dist passes over m-halves ---
        # PSUM accumulates 2*R[b,m] = 2*sum_d relu(memT[d,m] - encT[d,b]);
        # dist[b,m] = 2R - Sm[m] + Se[b]; loss[b] = min_m (2R - Sm) + Se[b].
        loss_parts = outsp.tile([BC, NPASS * (HALF // 512)], f32)

        # unit producer planner: greedily split relu-diff units across DVE
        # (tensor_scalar: 1127ns fp8 / 594ns bf16) and ACT (activation Relu
        # with bias=-encT: 1893ns) to balance the two engines.
        eng_t = {"dve": 12.5, "act": 11.2}  # initial busy estimates (us)
        UC = {("dve", True): 1.127, ("dve", False): 0.594, ("act", True): 1.893,
              ("act", False): 1.893}

        def produce(out_ap, h, b, p, is8):
            pick = min(
                ("dve", "act"),
                key=lambda e: max(
                    eng_t["dve"] + (UC[("dve", is8)] if e == "dve" else 0),
                    eng_t["act"] + (UC[("act", is8)] if e == "act" else 0),
                ),
            )
            eng_t[pick] += UC[(pick, is8)]
            src = memT[h][:, p * HALF : (p + 1) * HALF]
            if pick == "dve":
                nc.vector.tensor_scalar(
                    out_ap, src, encT_f32[:, h, b : b + 1], 0.0,
                    op0=Alu.subtract, op1=Alu.max,
                )
            else:
                nc.scalar.activation(
                    out_ap, src, Act.Relu,
                    bias=neg_encT[:, h, b : b + 1], scale=1.0,
                )

        for p in range(NPASS):
            dist = psum_dist.tile([BC, HALF], f32, tag="dist")
            for b in range(BC):
                if p == 0 and b == 0:
                    # prime the pipeline: sub-range absdiffs so the first PE
                    # matmuls start as soon as the first memT columns land
                    ad8 = absp.tile([128, 2, HALF], fp8e4, tag="ad8",
                                    name="ad8", bufs=3)
                    eng_t["dve"] += 2 * UC[("dve", True)]
                    for q in range(HALF // 512):
                        for h in range(2):
                            nc.vector.tensor_scalar(
                                ad8[:, h, q * 512 : (q + 1) * 512],
                                memT[h][:, q * 512 : (q + 1) * 512],
                                encT_f32[:, h, 0:1], 0.0,
                                op0=Alu.subtract, op1=Alu.max,
                            )
                        nc.tensor.matmul(
                            dist[:, q * 512 : (q + 1) * 512],
                            strip2[:, :, 31:63],
                            ad8[:, :, q * 512 : (q + 1) * 512],
                            start=True,
                            stop=False,
                            perf_mode=mybir.MatmulPerfMode.DoubleRow,
                        )
                    continue
                if b % 8 < NB8:
                    # fp8 path: both halves reduced by one DoubleRow matmul
                    ad8 = absp.tile([128, 2, HALF], fp8e4, tag="ad8",
                                    name="ad8", bufs=3)
                    for h in range(2):
                        produce(ad8[:, h, :], h, b, p, True)
                    for q in range(HALF // 512):
                        nc.tensor.matmul(
                            dist[:, q * 512 : (q + 1) * 512],
                            strip2[:, :, 31 - b : 63 - b],
                            ad8[:, :, q * 512 : (q + 1) * 512],
                            start=(b == 0),
                            stop=False,
                            perf_mode=mybir.MatmulPerfMode.DoubleRow,
                        )
                    continue
                for h in range(2):
                    ad = absp.tile([128, HALF], bf16, tag="ad")
                    produce(ad[:], h, b, p, False)
                    for q in range(HALF // 512):
                        nc.tensor.matmul(
                            dist[:, q * 512 : (q + 1) * 512],
                            strip[:, 31 - b : 63 - b],
                            ad[:, q * 512 : (q + 1) * 512],
                            start=False,
                            stop=False,
                        )
            # fold "- Sm[m]" into PSUM (dist[j,m] += -Sm[m]), then per-bank
            # min-reduce straight from PSUM
            for q in range(HALF // 512):
                nc.tensor.matmul(
                    dist[:, q * 512 : (q + 1) * 512],
                    negrow[:],
                    sm_sb[:, p * HALF + q * 512 : p * HALF + (q + 1) * 512],
                    start=False,
                    stop=True,
                )
            for q in range(HALF // 512):
                nc.vector.tensor_reduce(
                    loss_parts[:, p * (HALF // 512) + q : p * (HALF // 512) + q + 1],
                    dist[:, q * 512 : (q + 1) * 512],
                    axis=mybir.AxisListType.X, op=Alu.min,
                )
        loss_sb = outsp.tile([BC, 1], f32)
        nc.vector.tensor_reduce(
            loss_sb[:], loss_parts[:], axis=mybir.AxisListType.X, op=Alu.min
        )
        nc.vector.tensor_tensor(loss_sb[:], loss_sb[:], se_sb[:], op=Alu.add)
        nc.sync.dma_start(loss_out[:, :], loss_sb[:])

    nc.finalize()
    return nc


def run(inputs: dict, trace: bool = False):
    """Run the device part; returns (loss[256] f32, enc[256,256] f32, results)."""
    x = np.asarray(inputs["x"], np.float32)
    mean = np.asarray(inputs["mean"], np.float32)
    std = np.asarray(inputs["std"], np.float32)
    W1 = np.asarray(inputs["W1"], np.float32)
    b1 = np.asarray(inputs["b1"], np.float32)
    memory = np.asarray(inputs["memory"], np.float32)

    zero = std == 0
    inv = np.where(zero, 0.0, 1.0 / np.where(zero, 1.0, std)).astype(np.float32)
    W1p = np.ascontiguousarray((W1 * inv[:, None]).astype(np.float32))
    b1p = (b1 - (mean * inv) @ W1).astype(np.float32)
    b1p2 = np.ascontiguousarray(b1p.reshape(2, 128).T)
    xT = np.ascontiguousarray(x.T)  # [128, 256]

    if "nc" not in _CACHE:
        _CACHE["nc"] = _build_bass()
    nc = _CACHE["nc"]

    import ml_dtypes

    mem_bf = np.ascontiguousarray(memory.astype(ml_dtypes.bfloat16))
    sm = np.ascontiguousarray(
        memory.astype(ml_dtypes.bfloat16).astype(np.float32)
        .sum(axis=1, dtype=np.float32).reshape(1, M)
    ).astype(ml_dtypes.bfloat16)
    ident = np.eye(128, dtype=ml_dtypes.bfloat16)
    in_maps = []
    for c in range(NCORES):
        in_maps.append(
            {
                "xt": np.ascontiguousarray(xT[:, c * BC : (c + 1) * BC]),
                "w1p": W1p,
                "b1p2": b1p2,
                "identin": ident,
                "mem": mem_bf,
                "sm": sm,
            }
        )
    kw = {}
    if trace:
        kw = dict(trace=True, trace_cores=list(range(NCORES)))
    res = run_bass_kernel_spmd(nc, in_maps, core_ids=list(range(NCORES)), **kw)

    loss = np.concatenate([r["loss"][:, 0] for r in res.results])
    enc = np.concatenate(
        [r["enc_t"].reshape(256, BC).T for r in res.results], axis=0
    )
    return loss.astype(np.float32), enc.astype(np.float32), res


def kernel(x, mean, std, W1, b1, memory, mem_data):
    inputs = dict(x=x, mean=mean, std=std, W1=W1, b1=b1, memory=memory,
                  mem_data=mem_data)
    loss, enc, _ = run(inputs)

    memory = np.asarray(memory, np.float32)
    mem_data = np.asarray(mem_data, np.float32)
    x = np.asarray(x, np.float32)

    # sequential circular-buffer update on host (count starts at M == 0 mod M)
    cond = np.isfinite(loss) & (loss <= BETA)
    mem_out = memory.copy()
    md_out = mem_data.copy()
    if cond.any():
        pos = (np.cumsum(cond) - 1)[cond] % M
        mem_out[pos] = enc[cond]
        md_out[pos] = x[cond]
    return loss, mem_out, md_out
